# revision 13
# baseline (speedup 1.0000x reference)
"""Trainium2 Bass kernel for nn_CoeusMemoryLayer (gated-decay linear attention
+ local windowed attention + RMSNorm + output projection).

Sharding: heads are split across 8 cores (2 q-heads + their shared kv-head per
core).  Each core computes its head-group's output channels for all tokens,
then an AllToAll redistributes to token-sharding for the RMSNorm + c_proj,
whose row-shards ride in the same collective.  The global decay attention is
computed with an exact chunked (block-recurrent) reformulation: per 128-token
block, intra-block weights are computed densely and the inter-block
contribution flows through a [64,64] decayed state matrix.
"""

import numpy as np
from contextlib import ExitStack

B, T, C = 1, 2048, 1024
H, HKV, DH, WIN = 16, 4, 64, 128
EPS = 1e-5
NB = T // WIN          # 16 blocks of 128 tokens
NCORE = 8
USE_F32R = True        # fast fp32 (TF32-like) for the big matmuls

_CACHE = {}


def _build_program():
    import concourse.bacc as bacc
    import concourse.bass as bass
    import concourse.mybir as mybir
    import concourse.tile as tile

    f32 = mybir.dt.float32
    f32r = mybir.dt.float32r
    ALU = mybir.AluOpType
    AF = mybir.ActivationFunctionType

    def r(ap):
        return ap.bitcast(f32r) if USE_F32R else ap

    nc = bacc.Bacc("TRN2", target_bir_lowering=False, debug=False,
                   num_devices=NCORE)

    # ---- kernel I/O -----------------------------------------------------
    xT_d = nc.dram_tensor("xT", [C, T], f32r, kind="ExternalInput")
    wproj_d = nc.dram_tensor("wproj", [C, 258], f32r, kind="ExternalInput")
    bg_d = nc.dram_tensor("bg2", [128, 2], f32, kind="ExternalInput")
    cos_d = nc.dram_tensor("cos_t", [128, 512], f32, kind="ExternalInput")
    sin_d = nc.dram_tensor("sin_t", [128, 512], f32, kind="ExternalInput")
    mneg_d = nc.dram_tensor("mneg", [128, 128], f32, kind="ExternalInput")
    utneg_d = nc.dram_tensor("utneg", [128, 128], f32, kind="ExternalInput")
    i128_d = nc.dram_tensor("i128", [128, 128], f32, kind="ExternalInput")
    onescr_d = nc.dram_tensor("ones_cr", [128, 1], f32r, kind="ExternalInput")
    wcf_d = nc.dram_tensor("wcf", [C, C], f32r, kind="ExternalInput")
    out_d = nc.dram_tensor("out", [256, C], f32, kind="ExternalOutput")

    c_dram = nc.dram_tensor("c_dram", [128, 32], f32)
    a2a_in = nc.dram_tensor("a2a_in", [8, 129, 256], f32r)
    a2a_out = nc.dram_tensor("a2a_out", [8, 129, 256], f32r)

    def bc(ap, n, pos=1):
        """insert a step-0 broadcast free dim of count n at free position pos"""
        dims = list(ap.ap)
        dims.insert(pos, [0, n])
        return bass.AP(ap.tensor, ap.offset, dims)

    with tile.TileContext(nc) as tc, ExitStack() as ctx:
        # ---- persistent SBUF tensors -----------------------------------
        pers = ctx.enter_context(tc.tile_pool(name="pers", bufs=1))
        pj_ctx = ExitStack()
        pj_pool = pj_ctx.enter_context(tc.tile_pool(name="pjs", bufs=1))
        xT_sb = [pj_pool.tile([128, T], f32r, tag=f"xT{c}", name=f"xT{c}") for c in range(8)]
        wproj_sb = [pj_pool.tile([128, 258], f32r, tag=f"wp{c}", name=f"wp{c}") for c in range(8)]
        wcf_sb = [pers.tile([128, C], f32r, tag=f"wc{c}", name=f"wc{c}") for c in range(8)]
        cos_sb = pers.tile([128, 512], f32, tag="cos")
        sin_sb = pers.tile([128, 512], f32, tag="sin")
        mneg_sb = pers.tile([128, 128], f32, tag="mneg")
        utneg_sb = pers.tile([128, 128], f32, tag="utneg")
        i128_sb = pers.tile([128, 128], f32, tag="i128")
        bg_sb = pers.tile([128, 2], f32, tag="bg")
        ones_r = pers.tile([1, 128], f32, tag="ones_r")    # all ones row
        ones_c = pers.tile([128, 1], f32, tag="ones_c")    # all ones col
        negones_c = pers.tile([128, 1], f32, tag="negones_c")
        ones_cr = pers.tile([128, 1], f32r, tag="ones_cr")
        nones_r = pers.tile([1, 128], f32, tag="nones_r")

        q_sb = pers.tile([128, 16 * 128], f32, tag="q")    # [t, tile*128+feat]
        kv_sb = pers.tile([128, 16 * 128], f32, tag="kv")  # k: +0..63 v: +64..127
        z_sb = pers.tile([128, 32], f32, tag="z")
        zb_sb = pers.tile([128, 32], f32, tag="zb")
        spn_sb = pers.tile([128, 32], f32, tag="spn")      # softplus(-zb)
        lns_sb = pers.tile([128, 32], f32, tag="lns")      # log sigmoid(-zb)
        sig_sb = pers.tile([128, 32], f32, tag="sig")      # sigmoid(-zb)
        c_sb = pers.tile([128, 32], f32, tag="c")          # la_cum  [t, 2b+h]
        cT_sb = pers.tile([1, 4096], f32, tag="cT")        # row c per head
        carry_sb = pers.tile([1, 2], f32, tag="carry")
        brows_sb = pers.tile([1, 32], f32, tag="brows")    # boundary after blk
        bprev_sb = pers.tile([1, 32], f32, tag="bprev")    # boundary before blk
        u_sb = pers.tile([128, 32], f32, tag="u")          # cols 2b+h
        wq_sb = pers.tile([128, 32], f32, tag="wq")
        gam_sb = pers.tile([64, 32], f32, tag="gam")
        ss0_sb = pers.tile([64, 64], f32, tag="ss0")
        ss1_sb = pers.tile([64, 64], f32, tag="ss1")
        yT_sb = pers.tile([128, T], f32r, tag="yT")
        ssr_sb = pers.tile([1, T], f32r, tag="ssr")
        yTj_sb = [pers.tile([128, 256], f32r, tag=f"yTj{j}", name=f"yTj{j}") for j in range(8)]
        ss8_sb = pers.tile([8, 256], f32r, tag="ss8")
        inv_sb = pers.tile([1, 256], f32, tag="inv")
        sqr_sb = pers.tile([1, 256], f32, tag="sqr")
        invc_sb = pers.tile([128, 2], f32, tag="invc")
        eps_sb = pers.tile([1, 1], f32, tag="eps")

        # ---- input DMAs -------------------------------------------------
        for c in range(8):
            nc.sync.dma_start(out=xT_sb[c][:], in_=xT_d[128 * c:128 * (c + 1), :])
            nc.sync.dma_start(out=wproj_sb[c][:], in_=wproj_d[128 * c:128 * (c + 1), :])
            nc.sync.dma_start(out=wcf_sb[c][:], in_=wcf_d[128 * c:128 * (c + 1), :])
        nc.sync.dma_start(out=cos_sb[:], in_=cos_d[:])
        nc.sync.dma_start(out=sin_sb[:], in_=sin_d[:])
        nc.sync.dma_start(out=mneg_sb[:], in_=mneg_d[:])
        nc.sync.dma_start(out=utneg_sb[:], in_=utneg_d[:])
        nc.sync.dma_start(out=i128_sb[:], in_=i128_d[:])
        nc.sync.dma_start(out=bg_sb[:], in_=bg_d[:])
        nc.sync.dma_start(out=ones_cr[:], in_=onescr_d[:])
        nc.vector.memset(ones_r[:], 1.0)
        nc.vector.memset(ones_c[:], 1.0)
        nc.vector.memset(negones_c[:], -1.0)

        nc.vector.memset(nones_r[:], -1.0)
        nc.vector.memset(eps_sb[:], EPS)
        nc.vector.memset(ss0_sb[:], 0.0)
        nc.vector.memset(ss1_sb[:], 0.0)

        # ---- phase 1: projections --------------------------------------
        pj_psum = pj_ctx.enter_context(tc.tile_pool(name="pj", bufs=2, space="PSUM"))
        for tl in range(16):
            ps = pj_psum.tile([128, 258], f32, tag="proj")
            for cch in range(8):
                nc.tensor.matmul(
                    ps[:], xT_sb[cch][:, 128 * tl:128 * (tl + 1)],
                    wproj_sb[cch][:], start=(cch == 0), stop=(cch == 7))
            nc.vector.tensor_copy(q_sb[:, 128 * tl:128 * (tl + 1)], ps[:, 0:128])
            nc.scalar.copy(out=kv_sb[:, 128 * tl:128 * (tl + 1)], in_=ps[:, 128:256])
            nc.vector.tensor_copy(
                bass.AP(z_sb[:].tensor, z_sb[:].offset + tl, [[32, 128], [16, 2]]),
                ps[:, 256:258])

        # ---- phase 2: rope ---------------------------------------------
        # strided views [128, 16, 32]: lo/hi halves per group
        def halves(base, off):
            return bass.AP(base.tensor, base.offset + off,
                           [base.ap[0], [128, 16], [1, 32]])
        cosv = bass.AP(cos_sb[:].tensor, cos_sb[:].offset,
                       [cos_sb[:].ap[0], [32, 16], [1, 32]])
        sinv = bass.AP(sin_sb[:].tensor, sin_sb[:].offset,
                       [sin_sb[:].ap[0], [32, 16], [1, 32]])
        rp = pj_ctx.enter_context(tc.tile_pool(name="rope", bufs=1))
        for base, offs in ((q_sb[:], (0, 64)), (kv_sb[:], (0,))):
            for off in offs:
                lo, hi = halves(base, off), halves(base, off + 32)
                t1 = rp.tile([128, 512], f32, tag="t1")
                t2 = rp.tile([128, 512], f32, tag="t2")
                t3 = rp.tile([128, 512], f32, tag="t3")
                t4 = rp.tile([128, 512], f32, tag="t4")
                v1 = bass.AP(t1[:].tensor, t1[:].offset, [t1[:].ap[0], [32, 16], [1, 32]])
                v2 = bass.AP(t2[:].tensor, t2[:].offset, [t2[:].ap[0], [32, 16], [1, 32]])
                v3 = bass.AP(t3[:].tensor, t3[:].offset, [t3[:].ap[0], [32, 16], [1, 32]])
                v4 = bass.AP(t4[:].tensor, t4[:].offset, [t4[:].ap[0], [32, 16], [1, 32]])
                nc.vector.tensor_mul(v1, lo, cosv)
                nc.vector.tensor_mul(v2, hi, sinv)
                nc.vector.tensor_mul(v3, hi, cosv)
                nc.vector.tensor_mul(v4, lo, sinv)
                nc.vector.tensor_sub(lo, v1, v2)
                nc.vector.tensor_add(hi, v3, v4)

        # ---- phase 3: gate chain ---------------------------------------
        g_psum = pj_ctx.enter_context(tc.tile_pool(name="gps", bufs=3, space="PSUM"))
        nc.vector.tensor_add(
            bass.AP(zb_sb[:].tensor, zb_sb[:].offset, [[32, 128], [16, 2], [1, 16]]),
            bass.AP(z_sb[:].tensor, z_sb[:].offset, [[32, 128], [16, 2], [1, 16]]),
            bass.AP(bg_sb[:].tensor, bg_sb[:].offset, [[2, 128], [1, 2], [0, 16]]))
        # softplus(x) = ln(exp(x) + 1)  (Softplus has no ACT table here)
        nc.scalar.activation(spn_sb[:], zb_sb[:], AF.Exp, scale=-1.0)
        nc.scalar.activation(spn_sb[:], spn_sb[:], AF.Ln, bias=ones_c[:])
        nc.scalar.activation(lns_sb[:], zb_sb[:], AF.Exp, scale=1.0)
        nc.scalar.activation(lns_sb[:], lns_sb[:], AF.Ln, bias=ones_c[:])
        nc.vector.tensor_scalar_mul(lns_sb[:], lns_sb[:], -1.0)
        nc.scalar.activation(sig_sb[:], lns_sb[:], AF.Exp)
        # cumsum of -spn per head via UTneg matmul + running carry at partition 0
        nc.vector.memset(carry_sb[:], 0.0)
        for tl in range(16):
            cp = g_psum.tile([128, 2], f32, tag="g", name=f"cum{tl}")
            spn_t = bass.AP(spn_sb[:].tensor, spn_sb[:].offset + tl,
                            [[32, 128], [16, 2]])
            nc.tensor.matmul(cp[:], utneg_sb[:], spn_t,
                             start=True, stop=(tl == 0))
            if tl > 0:
                nc.tensor.matmul(cp[:], ones_r[:], carry_sb[:],
                                 start=False, stop=True, skip_group_check=True)
            nc.vector.tensor_copy(
                bass.AP(c_sb[:].tensor, c_sb[:].offset + tl, [[32, 128], [16, 2]]),
                cp[:])
            # carry += -sum(spn tile)   (negones_c.T @ spn)
            tt = g_psum.tile([1, 2], f32, tag="g", name=f"tt{tl}")
            nc.tensor.matmul(tt[:], negones_c[:], spn_t, start=True, stop=True)
            nc.vector.tensor_add(carry_sb[:], carry_sb[:], tt[:])
            # block-boundary cumsum value (= carry after this tile)
            nc.vector.tensor_copy(
                bass.AP(brows_sb[:].tensor, brows_sb[:].offset + tl,
                        [[32, 1], [16, 2]]),
                carry_sb[:])
        # bprev: boundary before each block (0 for block 0)
        nc.vector.memset(bprev_sb[0:1, 0:1], 0.0)
        nc.vector.memset(bprev_sb[0:1, 16:17], 0.0)
        nc.vector.tensor_copy(bprev_sb[0:1, 1:16], brows_sb[0:1, 0:15])
        nc.vector.tensor_copy(bprev_sb[0:1, 17:32], brows_sb[0:1, 16:31])
        # c in row layout per head: cT_sb[0, 2048*h + 128*b + p] = c_sb[p, 2b+h]
        nc.sync.dma_start(out=c_dram[:], in_=c_sb[:])
        nc.sync.dma_start(
            out=bass.AP(cT_sb[:].tensor, cT_sb[:].offset, [[4096, 1], [1, 4096]]),
            in_=bass.AP(c_dram[:].tensor, c_dram[:].offset, [[1, 32], [32, 128]]))
        # u / wq / gamma tables, both heads at once (cols 2b+h)
        bm = g_psum.tile([128, 32], f32, tag="g", name="bm")
        nc.tensor.matmul(bm[:], ones_r[:], brows_sb[:], start=True, stop=True)
        nc.vector.tensor_sub(u_sb[:], bm[:], c_sb[:])
        nc.scalar.activation(u_sb[:], u_sb[:], AF.Exp)
        nc.vector.tensor_mul(u_sb[:], u_sb[:], sig_sb[:])
        bpm = g_psum.tile([128, 32], f32, tag="g", name="bpm")
        nc.tensor.matmul(bpm[:], ones_r[:], bprev_sb[:], start=True, stop=True)
        nc.vector.tensor_sub(wq_sb[:], c_sb[:], bpm[:])
        nc.scalar.activation(wq_sb[:], wq_sb[:], AF.Exp)
        gb = pers.tile([1, 32], f32, tag="gb", name="gb")
        nc.vector.tensor_sub(gb[:], brows_sb[:], bprev_sb[:])
        nc.scalar.activation(gb[:], gb[:], AF.Exp)
        gmp = g_psum.tile([64, 32], f32, tag="g", name="gmp")
        nc.tensor.matmul(gmp[:], ones_r[0:1, 0:64], gb[:], start=True, stop=True)
        nc.vector.tensor_copy(gam_sb[:], gmp[:])

        # ---- phase 4: attention blocks ---------------------------------
        pj_ctx.close()
        blk_ctx = ExitStack()
        bp = blk_ctx.enter_context(tc.tile_pool(name="blk", bufs=3))
        p_big = blk_ctx.enter_context(tc.tile_pool(name="pbig", bufs=3, space="PSUM"))
        p_half = blk_ctx.enter_context(tc.tile_pool(name="phalf", bufs=5, space="PSUM"))
        p_den = p_half
        mnegb = bc(mneg_sb[:], 2)     # [128, (0,2), 128]
        for b in range(NB):
            q2 = q_sb[:, 128 * b:128 * (b + 1)]
            kblk = kv_sb[:, 128 * b:128 * b + 64]
            vblk = kv_sb[:, 128 * b + 64:128 * (b + 1)]
            # transposes
            qTp = p_half.tile([64, 256], f32, tag="h")
            nc.tensor.matmul(qTp[:, 0:128], q_sb[:, 128 * b:128 * b + 64],
                             i128_sb[:], is_transpose=True)
            nc.tensor.matmul(qTp[:, 128:256], q_sb[:, 128 * b + 64:128 * b + 128],
                             i128_sb[:], is_transpose=True)
            qT = bp.tile([64, 256], f32r, tag="qT")
            nc.scalar.copy(out=qT[:], in_=qTp[:])
            kTp = p_half.tile([64, 256], f32, tag="h")
            nc.tensor.matmul(kTp[:, 0:128], kblk, i128_sb[:], is_transpose=True)
            kT = bp.tile([64, 128], f32r, tag="kT")
            nc.scalar.copy(out=kT[:], in_=kTp[:, 0:128])
            qp2 = bp.tile([128, 128], f32, tag="qp2")
            nc.vector.tensor_scalar_mul(qp2[:, 0:64], q_sb[:, 128 * b:128 * b + 64],
                                        wq_sb[:, b:b + 1])
            nc.vector.tensor_scalar_mul(qp2[:, 64:128],
                                        q_sb[:, 128 * b + 64:128 * b + 128],
                                        wq_sb[:, 16 + b:16 + b + 1])
            qpTp = p_half.tile([64, 256], f32, tag="h")
            nc.tensor.matmul(qpTp[:, 0:128], qp2[:, 0:64], i128_sb[:], is_transpose=True)
            nc.tensor.matmul(qpTp[:, 128:256], qp2[:, 64:128], i128_sb[:], is_transpose=True)
            qpT = bp.tile([64, 256], f32, tag="qpT")
            nc.scalar.copy(out=qpT[:], in_=qpTp[:])
            # scores P = kT.T @ qT  -> [s, tq(2h)]
            Pp = p_big.tile([128, 256], f32, tag="big")
            nc.tensor.matmul(Pp[:], kT[:], qT[:], start=True, stop=True)
            Pm = bp.tile([128, 256], f32, tag="Pm")
            nc.vector.tensor_add(
                bass.AP(Pm[:].tensor, Pm[:].offset, [Pm[:].ap[0], [128, 2], [1, 128]]),
                bass.AP(Pp[:].tensor, Pp[:].offset, [Pp[:].ap[0], [128, 2], [1, 128]]),
                mnegb)
            E = bp.tile([128, 256], f32r, tag="E")
            nc.scalar.activation(E[:], Pm[:], AF.Exp, scale=0.125)
            # decay
            Dp = p_big.tile([128, 256], f32, tag="big")
            c_row2 = bass.AP(cT_sb[:].tensor, cT_sb[:].offset + 128 * b,
                             [[4096, 1], [2048, 2], [1, 128]])
            nc.tensor.matmul(Dp[:], ones_r[:], c_row2, start=True, stop=False)
            nc.tensor.matmul(Dp[:, 0:128], cT_sb[0:1, 128 * b:128 * (b + 1)],
                             nones_r[:], start=False, stop=True,
                             skip_group_check=True)
            nc.tensor.matmul(Dp[:, 128:256],
                             cT_sb[0:1, 2048 + 128 * b:2048 + 128 * (b + 1)],
                             nones_r[:], start=False, stop=True,
                             skip_group_check=True)
            Dm = bp.tile([128, 256], f32, tag="Dm")
            nc.vector.tensor_add(
                bass.AP(Dm[:].tensor, Dm[:].offset, [Dm[:].ap[0], [128, 2], [1, 128]]),
                bass.AP(Dp[:].tensor, Dp[:].offset, [Dp[:].ap[0], [128, 2], [1, 128]]),
                mnegb)
            D = bp.tile([128, 256], f32, tag="Dx")
            nc.scalar.activation(D[:, 0:128], Dm[:, 0:128], AF.Exp,
                                 bias=lns_sb[:, b:b + 1])
            nc.scalar.activation(D[:, 128:256], Dm[:, 128:256], AF.Exp,
                                 bias=lns_sb[:, 16 + b:17 + b])
            Wg = bp.tile([128, 256], f32, tag="Wg")
            nc.vector.tensor_mul(Wg[:], D[:], Pm[:])
            # local softmax denominator
            den = p_den.tile([1, 256], f32, tag="h")
            nc.tensor.matmul(den[:], ones_cr[:], E[:], start=True, stop=True)
            rc = bp.tile([1, 256], f32, tag="rc")
            nc.vector.reciprocal(rc[:], den[:])
            rcm = p_big.tile([128, 256], f32, tag="big")
            nc.tensor.matmul(rcm[:], ones_r[:], rc[:], start=True, stop=True)
            Wsum = bp.tile([128, 256], f32, tag="Wsum")
            nc.vector.tensor_mul(Wsum[:], E[:].bitcast(f32), rcm[:])
            nc.vector.tensor_add(Wsum[:], Wsum[:], Wg[:])
            # y^T block accumulation
            yp = p_half.tile([64, 256], f32, tag="h")
            nc.tensor.matmul(yp[:], vblk, Wsum[:], start=True, stop=False)
            nc.tensor.matmul(yp[:, 0:128], ss0_sb[:], qpT[:, 0:128], start=False,
                             stop=True, skip_group_check=True)
            nc.tensor.matmul(yp[:, 128:256], ss1_sb[:], qpT[:, 128:256], start=False,
                             stop=True, skip_group_check=True)
            nc.scalar.copy(out=yT_sb[0:64, 128 * b:128 * (b + 1)], in_=yp[:, 0:128])
            nc.scalar.copy(out=yT_sb[64:128, 128 * b:128 * (b + 1)], in_=yp[:, 128:256])
            # state update
            ku2 = bp.tile([128, 128], f32, tag="ku2")
            nc.vector.tensor_scalar_mul(ku2[:, 0:64], kblk, u_sb[:, b:b + 1])
            nc.vector.tensor_scalar_mul(ku2[:, 64:128], kblk, u_sb[:, 16 + b:17 + b])
            Up = p_half.tile([64, 128], f32, tag="h")
            nc.tensor.matmul(Up[:, 0:64], ku2[:, 0:64], vblk, start=True, stop=True)
            nc.tensor.matmul(Up[:, 64:128], ku2[:, 64:128], vblk, start=True, stop=True,
                             skip_group_check=True)
            nc.vector.scalar_tensor_tensor(ss0_sb[:], ss0_sb[:],
                                           gam_sb[:, b:b + 1], Up[:, 0:64],
                                           op0=ALU.mult, op1=ALU.add)
            nc.vector.scalar_tensor_tensor(ss1_sb[:], ss1_sb[:],
                                           gam_sb[:, 16 + b:17 + b], Up[:, 64:128],
                                           op0=ALU.mult, op1=ALU.add)

        # ---- phase 5: ss + ship ----------------------------------------
        blk_ctx.close()
        sp_ps = ctx.enter_context(tc.tile_pool(name="ssp", bufs=2, space="PSUM"))
        sp_sb = ctx.enter_context(tc.tile_pool(name="ssb", bufs=2))
        for n in range(4):
            ysq_t = sp_sb.tile([128, 512], f32r, tag="ysq")
            nc.scalar.activation(ysq_t[:], yT_sb[:, 512 * n:512 * (n + 1)].bitcast(f32), AF.Square)
            sps = sp_ps.tile([1, 512], f32, tag="ss")
            nc.tensor.matmul(sps[:], ones_cr[:], ysq_t[:],
                             start=True, stop=True)
            nc.scalar.copy(out=ssr_sb[0:1, 512 * n:512 * (n + 1)], in_=sps[:])
        for j in range(8):
            nc.sync.dma_start(out=a2a_in[j, 0:128, :],
                              in_=yT_sb[:, 256 * j:256 * (j + 1)])
        nc.sync.dma_start(
            out=a2a_in[:, 128, :],
            in_=bass.AP(ssr_sb[:].tensor, ssr_sb[:].offset, [[T, 1], [256, 8], [1, 256]]))

        nc.gpsimd.collective_compute(
            "AllToAll", ALU.bypass, replica_groups=[list(range(NCORE))],
            ins=[a2a_in[:]], outs=[a2a_out[:]])

        # ---- phase 6: rmsnorm + c_proj on my 256 tokens -----------------
        for j in range(8):
            nc.sync.dma_start(out=yTj_sb[j][:], in_=a2a_out[j, 0:128, :])
        nc.sync.dma_start(out=ss8_sb[:], in_=a2a_out[:, 128, :])
        cp_ps = ctx.enter_context(tc.tile_pool(name="cpp", bufs=2, space="PSUM"))
        sst = cp_ps.tile([1, 256], f32, tag="sst")
        nc.tensor.matmul(sst[:], ones_cr[0:8, :], ss8_sb[:], start=True, stop=True)
        nc.scalar.activation(sqr_sb[:], sst[:], AF.Sqrt, bias=eps_sb[:],
                             scale=1.0 / C)
        nc.vector.reciprocal(inv_sb[:], sqr_sb[:])
        for a in range(2):
            icp = cp_ps.tile([128, 1], f32, tag="icp")
            nc.tensor.matmul(icp[:], inv_sb[0:1, 128 * a:128 * (a + 1)],
                             i128_sb[0:1, 0:1], is_transpose=True)
            nc.vector.tensor_copy(invc_sb[:, a:a + 1], icp[:])
        op_ps = ctx.enter_context(tc.tile_pool(name="opp", bufs=2, space="PSUM"))
        ob = ctx.enter_context(tc.tile_pool(name="ob", bufs=2))
        for a in range(2):
            for n in range(2):
                ops = op_ps.tile([128, 512], f32, tag="o")
                for j in range(8):
                    nc.tensor.matmul(
                        ops[:], yTj_sb[j][:, 128 * a:128 * (a + 1)],
                        wcf_sb[j][:, 512 * n:512 * (n + 1)],
                        start=(j == 0), stop=(j == 7))
                osb = ob.tile([128, 512], f32, tag="osb")
                nc.vector.tensor_scalar_mul(osb[:], ops[:], invc_sb[:, a:a + 1])
                nc.sync.dma_start(
                    out=out_d[128 * a:128 * (a + 1), 512 * n:512 * (n + 1)],
                    in_=osb[:])

    nc.compile()
    return nc


def _host_inputs(x, Wq, Wk, Wv, Wc, Wg, bg, rms_w):
    f = np.float32
    x = np.asarray(x, f)
    xT = np.ascontiguousarray(x[0].T)                       # [C, T]
    inv_freq = (1.0 / (10000.0 ** (np.arange(0, DH, 2, dtype=f) / DH))).astype(f)
    frq = np.arange(T, dtype=f)[:, None] * inv_freq[None, :]     # [T, 32]
    cosf, sinf = np.cos(frq).astype(f), np.sin(frq).astype(f)
    # [128, tile*32 + j] layout
    cos_t = np.ascontiguousarray(cosf.reshape(16, 128, 32).transpose(1, 0, 2).reshape(128, 512))
    sin_t = np.ascontiguousarray(sinf.reshape(16, 128, 32).transpose(1, 0, 2).reshape(128, 512))
    mneg = np.tril(np.full((128, 128), -1e30, f), k=-1)
    utneg = -np.triu(np.ones((128, 128), f))
    i128 = np.eye(128, dtype=f)
    wcf = (np.asarray(rms_w, f)[:, None] * np.asarray(Wc, f)).astype(f)
    in_maps = []
    for g in range(NCORE):
        hkv = (2 * g) // 4
        wproj = np.concatenate([
            np.asarray(Wq, f)[:, 128 * g:128 * (g + 1)],
            np.asarray(Wk, f)[:, 64 * hkv:64 * (hkv + 1)],
            np.asarray(Wv, f)[:, 64 * hkv:64 * (hkv + 1)],
            np.asarray(Wg, f)[:, 2 * g:2 * g + 2]], axis=1)
        bg2 = np.tile(np.asarray(bg, f)[2 * g:2 * g + 2][None, :], (128, 1))
        in_maps.append({
            "xT": xT, "wproj": np.ascontiguousarray(wproj), "bg2": bg2,
            "cos_t": cos_t, "sin_t": sin_t, "mneg": mneg, "utneg": utneg,
            "i128": i128, "wcf": wcf, "ones_cr": np.ones((128, 1), f),
        })
    return in_maps


def _get_program():
    if "nc" not in _CACHE:
        _CACHE["nc"] = _build_program()
    return _CACHE["nc"]


def kernel(x, Wq, Wk, Wv, Wc, Wg, bg, rms_w):
    from concourse.bass_utils import run_bass_kernel_spmd
    nc = _get_program()
    in_maps = _host_inputs(x, Wq, Wk, Wv, Wc, Wg, bg, rms_w)
    res = run_bass_kernel_spmd(nc, in_maps, list(range(NCORE)))
    out = np.concatenate([res.results[g]["out"] for g in range(NCORE)], axis=0)
    return out[None].astype(np.float32)
